# revision 6
# baseline (speedup 1.0000x reference)
"""LDS forward kernel for Trainium2 (8 NeuronCores, data-parallel over batch).

Math: the reference LDS with diagonal A and d_in == 1 is an exact causal
convolution plus a batch-independent bias:
    out[b,t,o] = sum_{d=0}^{t} Ktot[d,o] * x[b,t-d] + bias[t,o]
    Ktot[d,o]  = sum_s B[s] A[s]^d C[s,o]  (+ M[o,0,d-1] for d in 1..KX)
    bias[t,o]  = sum_s h0[s] A[s]^{t+1} C[s,o]

Ktot is an exponential family in d -> numerically low rank. Host computes a
row-weighted (by sqrt(T-d), matching how often lag d is used) SVD in float64
and keeps rank R=32 (singular values vanish by 32): Ktot ~= Uf @ Vf. The bias
matrix gets its own rank-RB=32 SVD: bias ~= Ubf @ Vbf.

Device kernel per core (32 batch rows, 8 groups of 4 b-interleaved rows):
  stage1 (PE): w[b,t,j] = sum_d Uf[d,j] x[b,t-d]  -- blocked triangular
    Toeplitz matmul: moving operand is a shifted-window ("mega") view of the
    signal (128 SBUF partitions = 128 relative shifts, built by one
    half-height DMA + one DVE shift-copy to halve HBM reads), stationary is
    the reversed Uf lag-chunk [128, 32]. PSUM accumulates the lag chain.
  w eviction (DVE): PSUM -> w_sb bf16 [64, T*4]; rows 32:64 hold Ubf
    (bias left factor, DMA'd) so the bias add rides stage2's contraction.
  stage2 (PE): y[o, (t,b)] = chat[:, o-slab].T @ w_sb  with chat = [Vf; Vbf]
    stationary per 128-wide o-slab (LDWEIGHTS amortized across all tiles).
  y eviction (DVE+ACT split): PSUM fp32 -> bf16 staging [o, b, t];
    batched 512 KB stores to a [b, o, t] DRAM tensor (host transposes back).

Output travels as bf16 (error ~3e-4 vs the 2e-2 gate; halves the HBM-write
floor, which dominates at ~360 GB/s per core); host upcasts to fp32.
"""

import numpy as np
import ml_dtypes

BSZ, T, D_IN = 256, 512, 1
S, O, KX = 512, 512, 5
NCORES = 8
BLOC = BSZ // NCORES        # 32 batch rows per core
NBG = BLOC // 4             # 8 groups of 4 batch rows
XPW = 640                   # padded signal width: 127 zeros + 512 + 1 slack
MEGW = 576                  # mega tile width (seed cols 0:576; shifted +64)
R = 32                      # conv-kernel rank
RB = 32                     # bias rank
KC = R + RB                 # stage2 contraction

_prog_cache = {}
LAST_RESULTS = None         # BassKernelResults of the most recent run


def _build_program(n_bg):
    import concourse.bacc as bacc
    import concourse.bass as bass
    import concourse.mybir as mybir
    from concourse.tile import TileContext

    f32 = mybir.dt.float32
    bf16 = mybir.dt.bfloat16

    nc = bacc.Bacc("TRN2", target_bir_lowering=False, debug=False)
    # xint[g, i, b] = xpad[g*4 + b, i]  (b-interleaved padded signal)
    xint = nc.dram_tensor("xint", [n_bg, XPW, 4], bf16, kind="ExternalInput")
    urev = nc.dram_tensor("urev", [4, 128, R], bf16, kind="ExternalInput")
    chat = nc.dram_tensor("chat", [KC, O], bf16, kind="ExternalInput")
    ubint = nc.dram_tensor("ubint", [RB, T * 4], bf16, kind="ExternalInput")
    # out[b, o, t] (o-major; host transposes back to [b, t, o])
    out = nc.dram_tensor("out", [4 * n_bg, O, T], bf16, kind="ExternalOutput")

    with TileContext(nc) as tc:
        with (
            tc.tile_pool(name="consts", bufs=1) as cpool,
            tc.tile_pool(name="mega", bufs=n_bg) as mpool,
            tc.tile_pool(name="wsb", bufs=n_bg) as wpool,
            tc.tile_pool(name="ysb", bufs=4) as ypool,
            tc.tile_pool(name="ps1", bufs=2, space="PSUM") as p1pool,
            tc.tile_pool(name="ps2", bufs=4, space="PSUM") as p2pool,
        ):
            # Const loads + mega seed loads go on the scalar (ACT HWDGE)
            # ring (ACT's evictions only start later); output stores get the
            # sync (SP HWDGE) ring to themselves.
            urev_sb = cpool.tile([128, 4, R], bf16, tag="urev")
            nc.scalar.dma_start(out=urev_sb[:], in_=urev.ap().rearrange("d k j -> k d j"))
            chat_sb = cpool.tile([KC, O], bf16, tag="chat")
            nc.scalar.dma_start(out=chat_sb[:], in_=chat.ap())

            megas = []
            wsbs = []
            for bg in range(n_bg):
                # mega[k, tau, b] = xint[bg, tau + k, b]; seed the lower 64
                # partitions from HBM, then one DVE copy builds the upper 64
                # (mega[64+k, tau] = mega[k, tau+64]) -- halves HBM reads.
                mega = mpool.tile([128, MEGW, 4], bf16, tag="mega")
                src = bass.AP(xint, bg * XPW * 4, [[4, 64], [4, MEGW], [1, 4]])
                nc.scalar.dma_start(out=mega[0:64, :, :], in_=src)
                megas.append(mega)
                wsb = wpool.tile([KC, T * 4], bf16, tag="wsb")
                nc.scalar.dma_start(out=wsb[R:KC, :], in_=ubint.ap())
                wsbs.append(wsb)

            for bg in range(n_bg):
                if bg == 0:
                    nc.vector.tensor_copy(
                        out=megas[0][64:128, 0:512, :], in_=megas[0][0:64, 64:576, :]
                    )
                megaf = megas[bg][:, 0:512, :].rearrange("p t b -> p (t b)")
                wsb = wsbs[bg]
                # w_sb columns are (b, t)-ordered: c = b*T + t. Stage1 PSUM is
                # (tau, b)-ordered, so its eviction scatters; stage2/stores
                # then read contiguously.
                wv3 = wsb[0:R, :].rearrange("p (b t) -> p b t", b=4, t=T)
                # stage1: w^T[j, (t,b)] per 128-t tile, triangular lag chain
                for tci in range(4):
                    ps1 = p1pool.tile([R, 512], f32)
                    for dc in range(tci + 1):
                        q = tci - dc
                        nc.tensor.matmul(
                            ps1[:],
                            urev_sb[:, dc, :],
                            megaf[:, q * 512 : q * 512 + 512],
                            start=(dc == 0),
                            stop=(dc == tci),
                        )
                    nc.vector.tensor_copy(
                        out=wv3[:, :, tci * 128 : tci * 128 + 128].rearrange(
                            "p b t -> p t b"
                        ),
                        in_=ps1[:].rearrange("p (t b) -> p t b", t=128, b=4),
                    )
                if bg + 1 < n_bg:
                    nc.vector.tensor_copy(
                        out=megas[bg + 1][64:128, 0:512, :],
                        in_=megas[bg + 1][0:64, 64:576, :],
                    )
                # stage2: per o-slab, chat slice is the stationary operand;
                # one matmul per batch row (contiguous 512-t slice of w_sb)
                for slab in range(4):
                    ytile = ypool.tile([128, 4, T], bf16, tag="ysb")
                    for b in range(4):
                        ps2 = p2pool.tile([128, 512], f32)
                        nc.tensor.matmul(
                            ps2[:],
                            chat_sb[:, slab * 128 : slab * 128 + 128],
                            wsb[:, b * T : b * T + T],
                            start=True,
                            stop=True,
                        )
                        # evict [o, t] -> ytile[o, b, t] (contiguous)
                        if (slab * 4 + b) % 3 == 0:
                            nc.scalar.copy(out=ytile[:, b, :], in_=ps2[:])
                        else:
                            nc.vector.tensor_copy(out=ytile[:, b, :], in_=ps2[:])
                    # store: out[bg*4 + b, slab*128 + o_rel, t]; each
                    # descriptor run is 512 contiguous t's (1 KiB)
                    dstd = bass.AP(
                        out,
                        (bg * 4 * O + slab * 128) * T,
                        [[T, 128], [O * T, 4], [1, T]],
                    )
                    nc.sync.dma_start(out=dstd, in_=ytile[:])
    nc.compile()
    return nc


def _get_program(n_bg=NBG):
    if n_bg not in _prog_cache:
        _prog_cache[n_bg] = _build_program(n_bg)
    return _prog_cache[n_bg]


def host_prep(inputs, A, B, C, M, h0):
    """float64 host precompute: low-rank factors + padded interleaved signal."""
    x = inputs[:, :, 0].astype(np.float64)          # [BSZ, T]
    A64 = A.astype(np.float64)
    B64 = B.astype(np.float64)
    C64 = C.astype(np.float64)
    M64 = M.astype(np.float64)
    h64 = h0.astype(np.float64)

    Apow = A64[None, :] ** np.arange(T + 1)[:, None]      # [T+1, S]
    K = (B64[0][None, :] * Apow[:T]) @ C64                # [T, O]
    K[1 : KX + 1, :] += M64[:, 0, :].T                    # AR taps, lags 1..KX
    bias = (h64[None, :] * Apow[1 : T + 1]) @ C64         # [T, O]

    wts = np.sqrt(np.arange(T, 0, -1.0))                  # sqrt(T-d)
    U, sv, Vt = np.linalg.svd(wts[:, None] * K, full_matrices=False)
    Uf = (U[:, :R] * sv[:R]) / wts[:, None]               # [T, R]
    Vf = Vt[:R]                                           # [R, O]
    Ub, svb, Vbt = np.linalg.svd(bias, full_matrices=False)
    Ubf = Ub[:, :RB] * svb[:RB]                           # [T, RB]
    Vbf = Vbt[:RB]                                        # [RB, O]

    urev = np.ascontiguousarray(
        Uf.reshape(4, 128, R)[:, ::-1, :]
    ).astype(ml_dtypes.bfloat16)                          # [4, 128, R]
    chat = np.concatenate([Vf, Vbf], axis=0).astype(ml_dtypes.bfloat16)
    # ubint[i, b*T + t] = Ubf[t, i]  (w_sb columns are (b, t)-ordered)
    ubint = np.ascontiguousarray(
        np.tile(Ubf.T, (1, 4))
    ).astype(ml_dtypes.bfloat16)                          # [RB, T*4]

    xpad = np.zeros((BSZ, XPW), np.float32)
    xpad[:, 127 : 127 + T] = x
    xpad = xpad.astype(ml_dtypes.bfloat16)                # [BSZ, XPW]
    # xint[g, i, b] = xpad[g*4 + b, i]
    xint = np.ascontiguousarray(
        xpad.reshape(BSZ // 4, 4, XPW).transpose(0, 2, 1)
    )                                                     # [BSZ//4, XPW, 4]
    return xint, urev, chat, ubint


def kernel(inputs, A, B, C, M, h0):
    global LAST_RESULTS
    from concourse.bass_utils import run_bass_kernel_spmd

    xint, urev, chat, ubint = host_prep(inputs, A, B, C, M, h0)
    nc = _get_program(NBG)
    in_maps = [
        {
            "xint": np.ascontiguousarray(xint[c * NBG : (c + 1) * NBG]),
            "urev": urev,
            "chat": chat,
            "ubint": ubint,
        }
        for c in range(NCORES)
    ]
    res = run_bass_kernel_spmd(nc, in_maps, core_ids=list(range(NCORES)))
    LAST_RESULTS = res
    # results are [b, o, t] bf16; transpose to [b, t, o] and upcast
    y = np.concatenate([r["out"] for r in res.results], axis=0)
    return y.transpose(0, 2, 1).astype(np.float32)


# revision 8
# speedup vs baseline: 1.2448x; 1.2448x over previous
"""LDS forward kernel for Trainium2 (8 NeuronCores, data-parallel over batch).

Math: the reference LDS with diagonal A and d_in == 1 is an exact causal
convolution plus a batch-independent bias:
    out[b,t,o] = sum_{d=0}^{t} Ktot[d,o] * x[b,t-d] + bias[t,o]
    Ktot[d,o]  = sum_s B[s] A[s]^d C[s,o]  (+ M[o,0,d-1] for d in 1..KX)
    bias[t,o]  = sum_s h0[s] A[s]^{t+1} C[s,o]

Ktot is an exponential family in d -> numerically low rank. Host computes a
row-weighted (by sqrt(T-d), how often lag d is used) SVD in float64 and
keeps rank R=32 (singular values vanish by 32): Ktot ~= Uf @ Vf. The bias
matrix gets its own rank-RB=16 SVD: bias ~= Ubf @ Vbf.

Device kernel per core (32 batch rows, 8 groups of 4 rows):
  stage1 (PE): w[b,:,j] = conv(x[b], Uf[:,j]) as triangular lag-chunk
    matmuls; moving operand is a b-major shifted-window ("mega") view
    (128 partitions = 128 relative shifts; seeded half-height from HBM,
    upper half built by a shift-copy on GpSimd to halve HBM reads);
    stationary is the reversed Uf chunk [128, R]. PSUM [R, 1024] holds a
    row pair.
  w evict: PSUM -> w_sb bf16 [KC, 4*T] (contiguous); rows R:KC hold Ubf
    (bias left factor, DMA'd) so the bias add rides stage2's contraction.
  stage2 (PE): y[o_slab, t] = chat[:, slab].T @ w_sb per (slab, row);
    chat = [Vf; Vbf] is the stationary operand, amortized across tiles.
  evictions: single-instruction [., 1024] PSUM->bf16 casts (PSUM reads are
    1 elem/cycle; no 2x modes), split DVE/ACT to balance their clocks.
  stores: 1 MiB per (slab, group-pair) on the sync ring to a [b, o, t]
    DRAM tensor (host transposes back).

Output travels as bf16 (error ~3e-4 vs the 2e-2 gate; halves the HBM-write
floor, which dominates at ~360 GB/s per core); host upcasts to fp32.
"""

import numpy as np
import ml_dtypes

BSZ, T, D_IN = 256, 512, 1
S, O, KX = 512, 512, 5
NCORES = 8
BLOC = BSZ // NCORES        # 32 batch rows per core
NBG = BLOC // 4             # 8 groups of 4 batch rows
XPW = 640                   # padded signal width: 127 zeros + 512 + 1 slack
MEGW = 576                  # mega tile width (seed cols 0:576; shifted +64)
R = 32                      # conv-kernel rank
RB = 16                     # bias rank
KC = R + RB                 # stage2 contraction

_prog_cache = {}
LAST_RESULTS = None         # BassKernelResults of the most recent run


def _build_program(n_bg):
    import concourse.bacc as bacc
    import concourse.bass as bass
    import concourse.mybir as mybir
    from concourse.tile import TileContext

    f32 = mybir.dt.float32
    bf16 = mybir.dt.bfloat16

    nc = bacc.Bacc("TRN2", target_bir_lowering=False, debug=False)
    xpad = nc.dram_tensor("xpad", [4 * n_bg, XPW], bf16, kind="ExternalInput")
    urev = nc.dram_tensor("urev", [4, 128, R], bf16, kind="ExternalInput")
    chat = nc.dram_tensor("chat", [KC, O], bf16, kind="ExternalInput")
    ubint = nc.dram_tensor("ubint", [RB, 4 * T], bf16, kind="ExternalInput")
    # out[b, o, t] (o-major; host transposes back to [b, t, o])
    out = nc.dram_tensor("out", [4 * n_bg, O, T], bf16, kind="ExternalOutput")

    n_yunit = [0]

    def y_evict(dst, src):
        # DVE (0.96 GHz) : ACT (1.2 GHz) ~ 6 : 7 split of the y units
        if n_yunit[0] % 13 < 6:
            nc.vector.tensor_copy(out=dst, in_=src)
        else:
            nc.scalar.copy(out=dst, in_=src)
        n_yunit[0] += 1

    with TileContext(nc) as tc:
        with (
            tc.tile_pool(name="consts", bufs=1) as cpool,
            tc.tile_pool(name="mega", bufs=n_bg) as mpool,
            tc.tile_pool(name="wsb", bufs=n_bg) as wpool,
            tc.tile_pool(name="ysb", bufs=6) as ypool,
            tc.tile_pool(name="ps1", bufs=2, space="PSUM") as p1pool,
            tc.tile_pool(name="ps2", bufs=2, space="PSUM") as p2pool,
        ):
            # mega seeds + consts on the scalar (ACT HWDGE) ring; ubint on
            # sync; mega shift-copies on GpSimd; stores on sync.
            megas = []
            wsbs = []
            for bg in range(n_bg):
                # mega[k, b, c] = xpad[bg*4+b, c + k]; seed partitions 0:64
                # from HBM; the shift-copy builds 64:128
                # (mega[64+k, b, c] = mega[k, b, c+64]).
                mega = mpool.tile([128, 4, MEGW], bf16, tag="mega")
                src = bass.AP(xpad, bg * 4 * XPW, [[1, 64], [XPW, 4], [1, MEGW]])
                nc.scalar.dma_start(out=mega[0:64, :, :], in_=src)
                megas.append(mega)
                if bg == 0:
                    urev_sb = cpool.tile([128, 4, R], bf16, tag="urev")
                    nc.scalar.dma_start(
                        out=urev_sb[:], in_=urev.ap().rearrange("d k j -> k d j")
                    )
                    chat_sb = cpool.tile([KC, O], bf16, tag="chat")
                    nc.scalar.dma_start(out=chat_sb[:], in_=chat.ap())
                wsb = wpool.tile([KC, 4 * T], bf16, tag="wsb")
                nc.sync.dma_start(out=wsb[R:KC, :], in_=ubint.ap())
                wsbs.append(wsb)

            for pair in range(n_bg // 2):
                for g01 in range(2):
                    bg = pair * 2 + g01
                    nc.gpsimd.tensor_copy(
                        out=megas[bg][64:128, :, 0:512],
                        in_=megas[bg][0:64, :, 64:576],
                    )
                    wsb = wsbs[bg]
                    # stage1: psum [R, 1024] covers a pair of batch rows;
                    # lag chunk dc contributes to out cols tau >= dc*128,
                    # with the mega window always starting at c=0.
                    for bp in range(2):
                        ps1 = p1pool.tile([R, 1024], f32)
                        for bi in range(2):
                            b = bp * 2 + bi
                            for dc in range(4):
                                nc.tensor.matmul(
                                    ps1[:, bi * 512 + dc * 128 : bi * 512 + 512],
                                    urev_sb[:, dc, :],
                                    megas[bg][:, b, 0 : 512 - dc * 128],
                                    start=(dc == 0),
                                    stop=(dc == 3),
                                    skip_group_check=True,
                                )
                        dst = wsb[0:R, bp * 1024 : bp * 1024 + 1024]
                        if bp == 0:
                            nc.vector.tensor_copy(out=dst, in_=ps1[:])
                        else:
                            nc.scalar.copy(out=dst, in_=ps1[:])
                # stage2 for the pair, slab-major; staging covers both groups
                for slab in range(4):
                    yt = ypool.tile([128, 8, T], bf16, tag="ysb")
                    for g01 in range(2):
                        wsb = wsbs[pair * 2 + g01]
                        for bp in range(2):
                            ps2 = p2pool.tile([128, 1024], f32)
                            for bi in range(2):
                                b = bp * 2 + bi
                                nc.tensor.matmul(
                                    ps2[:, bi * 512 : bi * 512 + 512],
                                    chat_sb[:, slab * 128 : slab * 128 + 128],
                                    wsb[:, b * T : b * T + T],
                                    start=True,
                                    stop=True,
                                )
                            c0 = g01 * 4 + bp * 2
                            y_evict(
                                yt[:, c0 : c0 + 2, :].rearrange("p b t -> p (b t)"),
                                ps2[:],
                            )
                    # store: out[pair*8 + brow, slab*128 + o_rel, t]
                    dstd = bass.AP(
                        out,
                        (pair * 8 * O + slab * 128) * T,
                        [[T, 128], [O * T, 8], [1, T]],
                    )
                    nc.sync.dma_start(out=dstd, in_=yt[:])
    nc.compile()
    return nc


def _get_program(n_bg=NBG):
    if n_bg not in _prog_cache:
        _prog_cache[n_bg] = _build_program(n_bg)
    return _prog_cache[n_bg]


def host_prep(inputs, A, B, C, M, h0):
    """float64 host precompute: low-rank factors + padded signal."""
    x = inputs[:, :, 0].astype(np.float64)          # [BSZ, T]
    A64 = A.astype(np.float64)
    B64 = B.astype(np.float64)
    C64 = C.astype(np.float64)
    M64 = M.astype(np.float64)
    h64 = h0.astype(np.float64)

    Apow = A64[None, :] ** np.arange(T + 1)[:, None]      # [T+1, S]
    K = (B64[0][None, :] * Apow[:T]) @ C64                # [T, O]
    K[1 : KX + 1, :] += M64[:, 0, :].T                    # AR taps, lags 1..KX
    bias = (h64[None, :] * Apow[1 : T + 1]) @ C64         # [T, O]

    wts = np.sqrt(np.arange(T, 0, -1.0))                  # sqrt(T-d)
    U, sv, Vt = np.linalg.svd(wts[:, None] * K, full_matrices=False)
    Uf = (U[:, :R] * sv[:R]) / wts[:, None]               # [T, R]
    Vf = Vt[:R]                                           # [R, O]
    Ub, svb, Vbt = np.linalg.svd(bias, full_matrices=False)
    Ubf = Ub[:, :RB] * svb[:RB]                           # [T, RB]
    Vbf = Vbt[:RB]                                        # [RB, O]

    urev = np.ascontiguousarray(
        Uf.reshape(4, 128, R)[:, ::-1, :]
    ).astype(ml_dtypes.bfloat16)                          # [4, 128, R]
    chat = np.concatenate([Vf, Vbf], axis=0).astype(ml_dtypes.bfloat16)
    # ubint[i, b*T + t] = Ubf[t, i]  (w_sb columns are (b, t)-ordered)
    ubint = np.ascontiguousarray(
        np.tile(Ubf.T, (1, 4))
    ).astype(ml_dtypes.bfloat16)                          # [RB, 4*T]

    xpad = np.zeros((BSZ, XPW), np.float32)
    xpad[:, 127 : 127 + T] = x
    xpad = xpad.astype(ml_dtypes.bfloat16)                # [BSZ, XPW]
    return xpad, urev, chat, ubint


def kernel(inputs, A, B, C, M, h0):
    global LAST_RESULTS
    from concourse.bass_utils import run_bass_kernel_spmd

    xpad, urev, chat, ubint = host_prep(inputs, A, B, C, M, h0)
    nc = _get_program(NBG)
    in_maps = [
        {
            "xpad": np.ascontiguousarray(xpad[c * BLOC : (c + 1) * BLOC]),
            "urev": urev,
            "chat": chat,
            "ubint": ubint,
        }
        for c in range(NCORES)
    ]
    res = run_bass_kernel_spmd(nc, in_maps, core_ids=list(range(NCORES)))
    LAST_RESULTS = res
    # results are [b, o, t] bf16; transpose to [b, t, o] and upcast
    y = np.concatenate([r["out"] for r in res.results], axis=0)
    return y.transpose(0, 2, 1).astype(np.float32)


# revision 9
# speedup vs baseline: 1.3452x; 1.0806x over previous
"""LDS forward kernel for Trainium2 (8 NeuronCores, data-parallel over batch).

Math: the reference LDS with diagonal A and d_in == 1 is an exact causal
convolution plus a batch-independent bias:
    out[b,t,o] = sum_{d=0}^{t} Ktot[d,o] * x[b,t-d] + bias[t,o]
    Ktot[d,o]  = sum_s B[s] A[s]^d C[s,o]  (+ M[o,0,d-1] for d in 1..KX)
    bias[t,o]  = sum_s h0[s] A[s]^{t+1} C[s,o]

Ktot is an exponential family in d -> numerically low rank. Host computes a
row-weighted (by sqrt(T-d), how often lag d is used) SVD in float64 and
keeps rank R=32 (singular values vanish by 32): Ktot ~= Uf @ Vf. The bias
matrix gets its own rank-RB=16 SVD: bias ~= Ubf @ Vbf.

Device kernel per core (32 batch rows, 8 groups of 4 rows):
  stage1 (PE): w[b,:,j] = conv(x[b], Uf[:,j]) as triangular lag-chunk
    matmuls; moving operand is a b-major shifted-window ("mega") view
    (128 partitions = 128 relative shifts; seeded half-height from HBM,
    upper half built by a shift-copy on GpSimd to halve HBM reads);
    stationary is the reversed Uf chunk [128, R]. PSUM [R, 1024] holds a
    row pair.
  w evict: PSUM -> w_sb bf16 [KC, 4*T] (contiguous); rows R:KC hold Ubf
    (bias left factor, DMA'd) so the bias add rides stage2's contraction.
  stage2 (PE): y[o_slab, t] = chat[:, slab].T @ w_sb per (slab, row);
    chat = [Vf; Vbf] is the stationary operand, amortized across tiles.
  evictions: single-instruction [., 1024] PSUM->bf16 casts (PSUM reads are
    1 elem/cycle; no 2x modes), split DVE/ACT to balance their clocks.
  stores: 1 MiB per (slab, group-pair) on the sync ring to a [b, o, t]
    DRAM tensor (host transposes back).

Output travels as bf16 (error ~3e-4 vs the 2e-2 gate; halves the HBM-write
floor, which dominates at ~360 GB/s per core); host upcasts to fp32.
"""

import numpy as np
import ml_dtypes

BSZ, T, D_IN = 256, 512, 1
S, O, KX = 512, 512, 5
NCORES = 8
BLOC = BSZ // NCORES        # 32 batch rows per core
NBG = BLOC // 4             # 8 groups of 4 batch rows
XPW = 640                   # padded signal width: 127 zeros + 512 + 1 slack
MEGW = 576                  # mega tile width (seed cols 0:576; shifted +64)
R = 32                      # conv-kernel rank
TRUNC = 256                 # conv truncated at this lag (tail rel-err 5e-3)
NDC = TRUNC // 128          # lag chunks
RB = 16                     # bias rank
KC = R + RB                 # stage2 contraction

_prog_cache = {}
LAST_RESULTS = None         # BassKernelResults of the most recent run


def _build_program(n_bg):
    import concourse.bacc as bacc
    import concourse.bass as bass
    import concourse.mybir as mybir
    from concourse.tile import TileContext

    f32 = mybir.dt.float32
    bf16 = mybir.dt.bfloat16

    nc = bacc.Bacc("TRN2", target_bir_lowering=False, debug=False)
    xpad = nc.dram_tensor("xpad", [4 * n_bg, XPW], bf16, kind="ExternalInput")
    urev = nc.dram_tensor("urev", [NDC, 128, R], bf16, kind="ExternalInput")
    chat = nc.dram_tensor("chat", [KC, O], bf16, kind="ExternalInput")
    ubint = nc.dram_tensor("ubint", [RB, 4 * T], bf16, kind="ExternalInput")
    # out[b, o, t] (o-major; host transposes back to [b, t, o])
    out = nc.dram_tensor("out", [4 * n_bg, O, T], bf16, kind="ExternalOutput")

    n_yunit = [0]

    def y_evict(dst, src):
        # DVE (0.96 GHz) : ACT (1.2 GHz) ~ 6 : 7 split of the y units
        if n_yunit[0] % 13 < 6:
            nc.vector.tensor_copy(out=dst, in_=src)
        else:
            nc.scalar.copy(out=dst, in_=src)
        n_yunit[0] += 1

    with TileContext(nc) as tc:
        with (
            tc.tile_pool(name="consts", bufs=1) as cpool,
            tc.tile_pool(name="mega", bufs=n_bg) as mpool,
            tc.tile_pool(name="wsb", bufs=n_bg) as wpool,
            tc.tile_pool(name="ysb", bufs=6) as ypool,
            tc.tile_pool(name="ps1", bufs=2, space="PSUM") as p1pool,
            tc.tile_pool(name="ps2", bufs=2, space="PSUM") as p2pool,
        ):
            # mega seeds + consts on the scalar (ACT HWDGE) ring; ubint on
            # sync; mega shift-copies on GpSimd; stores on sync.
            megas = []
            wsbs = []
            for bg in range(n_bg):
                # mega[k, b, c] = xpad[bg*4+b, c + k]; seed partitions 0:64
                # from HBM; the shift-copy builds 64:128
                # (mega[64+k, b, c] = mega[k, b, c+64]).
                mega = mpool.tile([128, 4, MEGW], bf16, tag="mega")
                src = bass.AP(xpad, bg * 4 * XPW, [[1, 64], [XPW, 4], [1, MEGW]])
                nc.scalar.dma_start(out=mega[0:64, :, :], in_=src)
                megas.append(mega)
                if bg == 0:
                    urev_sb = cpool.tile([128, NDC, R], bf16, tag="urev")
                    nc.scalar.dma_start(
                        out=urev_sb[:], in_=urev.ap().rearrange("d k j -> k d j")
                    )
                    chat_sb = cpool.tile([KC, O], bf16, tag="chat")
                    nc.scalar.dma_start(out=chat_sb[:], in_=chat.ap())
                wsb = wpool.tile([KC, 4 * T], bf16, tag="wsb")
                nc.sync.dma_start(out=wsb[R:KC, :], in_=ubint.ap())
                wsbs.append(wsb)

            for pair in range(n_bg // 2):
                for g01 in range(2):
                    bg = pair * 2 + g01
                    nc.gpsimd.dma_start(
                        out=megas[bg][64:128, :, 0:512],
                        in_=megas[bg][0:64, :, 64:576],
                    )
                    wsb = wsbs[bg]
                    # stage1: psum [R, 1024] covers a pair of batch rows;
                    # lag chunk dc contributes to out cols tau >= dc*128,
                    # with the mega window always starting at c=0.
                    for bp in range(2):
                        ps1 = p1pool.tile([R, 1024], f32)
                        for bi in range(2):
                            b = bp * 2 + bi
                            for dc in range(NDC):
                                nc.tensor.matmul(
                                    ps1[:, bi * 512 + dc * 128 : bi * 512 + 512],
                                    urev_sb[:, dc, :],
                                    megas[bg][:, b, 0 : 512 - dc * 128],
                                    start=(dc == 0),
                                    stop=(dc == NDC - 1),
                                    skip_group_check=True,
                                )
                        dst = wsb[0:R, bp * 1024 : bp * 1024 + 1024]
                        if bp == 0:
                            nc.vector.tensor_copy(out=dst, in_=ps1[:])
                        else:
                            nc.scalar.copy(out=dst, in_=ps1[:])
                # stage2 for the pair, slab-major; staging covers both groups
                for slab in range(4):
                    yt = ypool.tile([128, 8, T], bf16, tag="ysb")
                    for g01 in range(2):
                        wsb = wsbs[pair * 2 + g01]
                        for bp in range(2):
                            ps2 = p2pool.tile([128, 1024], f32)
                            for bi in range(2):
                                b = bp * 2 + bi
                                nc.tensor.matmul(
                                    ps2[:, bi * 512 : bi * 512 + 512],
                                    chat_sb[:, slab * 128 : slab * 128 + 128],
                                    wsb[:, b * T : b * T + T],
                                    start=True,
                                    stop=True,
                                )
                            c0 = g01 * 4 + bp * 2
                            y_evict(
                                yt[:, c0 : c0 + 2, :].rearrange("p b t -> p (b t)"),
                                ps2[:],
                            )
                    # store: out[pair*8 + brow, slab*128 + o_rel, t]
                    dstd = bass.AP(
                        out,
                        (pair * 8 * O + slab * 128) * T,
                        [[T, 128], [O * T, 8], [1, T]],
                    )
                    nc.sync.dma_start(out=dstd, in_=yt[:])
    nc.compile()
    return nc


def _get_program(n_bg=NBG):
    if n_bg not in _prog_cache:
        _prog_cache[n_bg] = _build_program(n_bg)
    return _prog_cache[n_bg]


def host_prep(inputs, A, B, C, M, h0):
    """float64 host precompute: low-rank factors + padded signal."""
    x = inputs[:, :, 0].astype(np.float64)          # [BSZ, T]
    A64 = A.astype(np.float64)
    B64 = B.astype(np.float64)
    C64 = C.astype(np.float64)
    M64 = M.astype(np.float64)
    h64 = h0.astype(np.float64)

    Apow = A64[None, :] ** np.arange(T + 1)[:, None]      # [T+1, S]
    K = (B64[0][None, :] * Apow[:T]) @ C64                # [T, O]
    K[1 : KX + 1, :] += M64[:, 0, :].T                    # AR taps, lags 1..KX
    bias = (h64[None, :] * Apow[1 : T + 1]) @ C64         # [T, O]

    wts = np.sqrt(np.arange(T, 0, -1.0))                  # sqrt(T-d)
    U, sv, Vt = np.linalg.svd(
        (wts[:, None] * K)[:TRUNC], full_matrices=False
    )
    Uf = (U[:, :R] * sv[:R]) / wts[:TRUNC, None]          # [TRUNC, R]
    Vf = Vt[:R]                                           # [R, O]
    Ub, svb, Vbt = np.linalg.svd(bias, full_matrices=False)
    Ubf = Ub[:, :RB] * svb[:RB]                           # [T, RB]
    Vbf = Vbt[:RB]                                        # [RB, O]

    urev = np.ascontiguousarray(
        Uf.reshape(NDC, 128, R)[:, ::-1, :]
    ).astype(ml_dtypes.bfloat16)                          # [NDC, 128, R]
    chat = np.concatenate([Vf, Vbf], axis=0).astype(ml_dtypes.bfloat16)
    # ubint[i, b*T + t] = Ubf[t, i]  (w_sb columns are (b, t)-ordered)
    ubint = np.ascontiguousarray(
        np.tile(Ubf.T, (1, 4))
    ).astype(ml_dtypes.bfloat16)                          # [RB, 4*T]

    xpad = np.zeros((BSZ, XPW), np.float32)
    xpad[:, 127 : 127 + T] = x
    xpad = xpad.astype(ml_dtypes.bfloat16)                # [BSZ, XPW]
    return xpad, urev, chat, ubint


def kernel(inputs, A, B, C, M, h0):
    global LAST_RESULTS
    from concourse.bass_utils import run_bass_kernel_spmd

    xpad, urev, chat, ubint = host_prep(inputs, A, B, C, M, h0)
    nc = _get_program(NBG)
    in_maps = [
        {
            "xpad": np.ascontiguousarray(xpad[c * BLOC : (c + 1) * BLOC]),
            "urev": urev,
            "chat": chat,
            "ubint": ubint,
        }
        for c in range(NCORES)
    ]
    res = run_bass_kernel_spmd(nc, in_maps, core_ids=list(range(NCORES)))
    LAST_RESULTS = res
    # results are [b, o, t] bf16; transpose to [b, t, o] and upcast
    y = np.concatenate([r["out"] for r in res.results], axis=0)
    return y.transpose(0, 2, 1).astype(np.float32)
